# revision 36
# baseline (speedup 1.0000x reference)
"""Trainium2 Bass kernel for nn_L4maAttention (llama3.1-style GQA attention layer).

Sharding: heads across 8 cores (4 Q heads + 1 KV head per core).
  - full hidden_states shipped (bf16, transposed) to every core from host;
    no on-device h AllGather -> projections start immediately.
  - q/k/v projections column-parallel + rope on device. Single-pass PSUM
    accumulation over the full 4096 contraction (quarter-token accumulator
    tiles), evicted via scalar copy + rope on vector/gpsimd.
  - paged-KV context gathered on host, shipped pre-transposed per core
    (1 kv head per core, fp8 in DRAM, bf16 in SBUF via dge-cast).
  - attention per-head local in S^T layout ([kv, q]); 2-head groups
    (1024-wide tiles); softmax denominators via ones-matmul accumulated in
    PSUM; reciprocal_approx_fast; masks added on DVE for the 4 new-kv chunks.
  - o_proj COLUMN-parallel: per-batch AllGather of bf16 attention outputs,
    each core computes a disjoint 512-column slice. Output bf16 [512, 2048].

All matmul operands bf16 (fp8 beyond the kv-ctx DRAM encoding fails the
2e-2 accuracy budget; measured on host).
"""

import math
import sys

import numpy as np

sys.path.insert(0, "/opt/trn_rl_repo")

import concourse.bass as bass  # noqa: E402
import concourse.mybir as mybir  # noqa: E402
import concourse.tile as tile  # noqa: E402
from concourse import bacc  # noqa: E402
from concourse.bass_utils import run_bass_kernel_spmd  # noqa: E402
from concourse.masks import make_identity  # noqa: E402

import ml_dtypes  # noqa: E402

# ---- problem constants (hardcoded from spec) ----
B, QO, PAGE = 4, 512, 16
HID, HQ, HKV, D = 4096, 32, 8, 128
N = B * QO  # 2048
NCORES = 8
HQL = HQ // NCORES  # 4 local q heads
ROPE_THETA = 5e5
OLD_CTX, LOW_F, HIGH_F, RSCALE = 8192.0, 1.0, 4.0, 8.0
SM_SCALE = 1.0 / math.sqrt(D)

BF16NP = ml_dtypes.bfloat16
FP8NP = ml_dtypes.float8_e4m3
F32 = mybir.dt.float32
BF16 = mybir.dt.bfloat16
FP8 = mybir.dt.float8e4
AF = mybir.ActivationFunctionType
ALU = mybir.AluOpType
P = 128
KH = HID // P  # 32 contraction chunks


def _llama31_inv_freq(d):
    inv = ROPE_THETA ** (-np.arange(0, d, 2, dtype=np.float32) / d)
    wavelen = 2.0 * np.pi / inv
    low_wl, high_wl = OLD_CTX / LOW_F, OLD_CTX / HIGH_F
    smooth = (OLD_CTX / wavelen - LOW_F) / (HIGH_F - LOW_F)
    mid = (1.0 - smooth) * inv / RSCALE + smooth * inv
    return np.where(
        wavelen > low_wl, inv / RSCALE, np.where(wavelen < high_wl, inv, mid)
    ).astype(np.float32)


def host_prep(inputs):
    """Shard + pre-transpose inputs for the 8 cores. Returns (in_maps, ctxl)."""
    hs = np.asarray(inputs["hidden_states"], np.float32)
    pos_ids = np.asarray(inputs["position_ids"], np.int32)
    kvc = np.asarray(inputs["kv_cache"], np.float32)
    kpi = np.asarray(inputs["kv_page_indices"], np.int32)
    kpp = np.asarray(inputs["kv_page_indptr"], np.int32)
    klp = np.asarray(inputs["kv_last_page_lens"], np.int32)
    qop = np.asarray(inputs["qo_indptr"], np.int32)
    Wq = np.asarray(inputs["Wq"], np.float32)
    Wk = np.asarray(inputs["Wk"], np.float32)
    Wv = np.asarray(inputs["Wv"], np.float32)
    Wo = np.asarray(inputs["Wo"], np.float32)

    n, hid = hs.shape
    b_sz = qop.shape[0] - 1
    qo_len = n // b_sz
    page = kvc.shape[2]
    pps = kpi.shape[0] // b_sz
    seq_len = (pps - 1) * page + klp  # [B]
    ctx_len = seq_len - qo_len
    assert n == N and hid == HID and b_sz == B and qo_len == QO
    assert np.all(ctx_len == ctx_len[0]) and int(ctx_len[0]) % 128 == 0
    ctxl = int(ctx_len[0])

    # rope tables [64, N] indexed (freq, token)
    inv = _llama31_inv_freq(D)
    ang = pos_ids.astype(np.float32)[:, None] * inv[None, :]
    cosT = np.ascontiguousarray(np.cos(ang).T).astype(BF16NP)
    sinT = np.ascontiguousarray(np.sin(ang).T).astype(BF16NP)

    # gather paged KV context (positions 0..ctxl-1 per sequence)
    cpos = np.arange(ctxl)
    pages = kpi[kpp[:-1][:, None] + (cpos[None, :] // page)]  # [B, ctxl]
    slots = np.broadcast_to(cpos % page, (b_sz, ctxl))
    Kc = kvc[pages, 0, slots]  # [B, ctxl, HKV, D]
    Vc = kvc[pages, 1, slots]

    # per-chunk causal mask for the new-kv block, [128, 4*512]:
    # chunk jn holds rows kv_rel in [jn*128,(jn+1)*128) vs all 512 q_rel cols
    qr = np.arange(qo_len)
    mbig = np.where(qr[:, None] <= qr[None, :], 0.0, -1e30).astype(np.float32)
    msk = np.ascontiguousarray(
        np.concatenate(
            [mbig[i * 128 : (i + 1) * 128] for i in range(qo_len // 128)], axis=1
        )
    ).astype(BF16NP)
    hTf = hs.T.astype(BF16NP)  # [HID, N]
    # p-major: [128, kt*N + n] so every device DMA is contiguous wide lines
    hP = np.ascontiguousarray(
        hTf.reshape(HID // 128, 128, N).transpose(1, 0, 2).reshape(128, -1))

    Wq4 = Wq.reshape(HQ, D, HID)
    Wk4 = Wk.reshape(HKV, D, HID)
    Wv4 = Wv.reshape(HKV, D, HID)

    in_maps = []
    for i in range(NCORES):
        def pmaj(wT):  # [HID, X] -> [128, kt*X + x]
            x = wT.shape[1]
            return np.ascontiguousarray(
                wT.reshape(HID // 128, 128, x).transpose(1, 0, 2).reshape(128, -1))
        wqT = pmaj(Wq4[i * HQL : (i + 1) * HQL].reshape(HQL * D, HID).T.astype(BF16NP))
        wkT = pmaj(Wk4[i].T.astype(BF16NP))
        wvT = pmaj(Wv4[i].T.astype(BF16NP))
        # column-parallel o_proj slice: out cols [i*512,(i+1)*512)
        woTc = pmaj(Wo[i * QO : (i + 1) * QO, :].T.astype(BF16NP))
        kctxT = np.ascontiguousarray(
            Kc[:, :, i, :].reshape(b_sz * ctxl, D).T
        ).astype(FP8NP)
        vctx = np.ascontiguousarray(
            Vc[:, :, i, :].reshape(-1, 128, D).transpose(1, 0, 2).reshape(128, b_sz * ctxl)
        ).astype(FP8NP)
        in_maps.append(
            dict(hT=hP, wqT=wqT, wkT=wkT, wvT=wvT, woTc=woTc, kctxT=kctxT,
                 vctx=vctx, cosT=cosT, sinT=sinT, msk=msk)
        )
    return in_maps, ctxl


def build_program(ctxl):
    KVL = ctxl + QO  # kv length per sequence (2048)
    CC = ctxl // 128  # context chunks per sequence (12)
    KC = KVL // 128  # total kv chunks per sequence (16)

    nc = bacc.Bacc("TRN2", debug=False, num_devices=NCORES)
    hT = nc.dram_tensor("hT", [P, KH * N], BF16, kind="ExternalInput").ap()
    wqT = nc.dram_tensor("wqT", [P, KH * HQL * D], BF16, kind="ExternalInput").ap()
    wkT = nc.dram_tensor("wkT", [P, KH * D], BF16, kind="ExternalInput").ap()
    wvT = nc.dram_tensor("wvT", [P, KH * D], BF16, kind="ExternalInput").ap()
    woTc = nc.dram_tensor("woTc", [P, KH * QO], BF16, kind="ExternalInput").ap()
    kctxT = nc.dram_tensor("kctxT", [D, B * ctxl], FP8, kind="ExternalInput").ap()
    vctx = nc.dram_tensor("vctx", [P, B * ctxl], FP8, kind="ExternalInput").ap()
    cosT = nc.dram_tensor("cosT", [D // 2, N], BF16, kind="ExternalInput").ap()
    sinT = nc.dram_tensor("sinT", [D // 2, N], BF16, kind="ExternalInput").ap()
    msk = nc.dram_tensor("msk", [P, (QO // 128) * QO], BF16, kind="ExternalInput").ap()
    outT = nc.dram_tensor("outT", [QO, N], BF16, kind="ExternalOutput").ap()
    ones_c = nc.inline_tensor(np.ones((P, P), BF16NP), name="ones_c").ap()

    rg = [list(range(NCORES))]

    with tile.TileContext(nc) as tc:
        with tc.tile_pool(name="dram", bufs=1, space="DRAM") as dram:
            # tiny warm-up AllGather to absorb the first-collective barrier
            # while projections run
            wu_in = dram.tile([P, 8], BF16, name="wu_in")
            wu_out = dram.tile([NCORES * P, 8], BF16, addr_space="Shared", name="wu_out")
            nc.gpsimd.collective_compute(
                "AllGather", ALU.bypass, replica_groups=rg,
                ins=[wu_in[:]], outs=[wu_out[:]])
            # per-batch O all-gather buffers
            oins = [dram.tile([HQL * D, QO], BF16, tag=f"oin{b}", name=f"oin{b}")
                    for b in range(B)]
            oouts = [dram.tile([HQ * D, QO], BF16, addr_space="Shared",
                               tag=f"oout{b}", name=f"oout{b}") for b in range(B)]

            with tc.tile_pool(name="resident", bufs=1) as res:
                # q_sb: batch-major: col = b*2048 + m*512 + q
                q_sb = res.tile([P, HQL * N], BF16)
                kn_sb = res.tile([P, N], BF16)   # new K^T: [d, global token]
                vn_sb = res.tile([P, N], BF16)   # new V: 128-block t at cols t*128
                cos_sb = res.tile([P, N], BF16)
                sin_sb = res.tile([P, N], BF16)
                kctx_sb = res.tile([P, B * ctxl], BF16)
                vctx_sb = res.tile([P, B * ctxl], BF16)
                msk_sb = res.tile([P, (QO // 128) * QO], BF16)
                ones_sb = res.tile([P, P], BF16)
                ident = res.tile([P, P], BF16)
                make_identity(nc, ident[:])

                # ============ Phase A: QKV projections + rope ============
                with tc.tile_pool(name="wsb", bufs=1) as wpool, \
                     tc.tile_pool(name="hstream", bufs=18) as hpool, \
                     tc.tile_pool(name="evtmp", bufs=2) as epool, \
                     tc.tile_pool(name="apsum", bufs=6, space="PSUM") as apool, \
                     tc.tile_pool(name="tpsum", bufs=2, space="PSUM") as tpool:
                    wq_sb = wpool.tile([P, KH * HQL * D], BF16)  # (kt,m) at kt*512+m*128
                    wk_sb = wpool.tile([P, KH * D], BF16)
                    wv_sb = wpool.tile([P, KH * D], BF16)
                    # batched first-wave DMAs (few triggers, sized so the
                    # first matmuls start fast): h in 2-kt chunks on gpsimd,
                    # wq split sync/scalar, wv/wk halves first on scalar.
                    hTv = hT.rearrange("p (k n) -> p k n", n=N)
                    nc.gpsimd.dma_start(wv_sb[:, 0:1024], wvT[:, 0:1024])
                    nc.scalar.dma_start(wk_sb[:, 0:1024], wkT[:, 0:1024])
                    nc.sync.dma_start(wq_sb[:, 0:1024], wqT[:, 0:1024])
                    for c in range(1, 4):
                        s = slice(c * 1024, (c + 1) * 1024)
                        nc.scalar.dma_start(wv_sb[:, s], wvT[:, s])
                        nc.scalar.dma_start(wk_sb[:, s], wkT[:, s])
                    hts = {}
                    for c in range(16):
                        t = hpool.tile([P, 2 * 1024], BF16, tag="h", name=f"h0_{c}")
                        nc.gpsimd.dma_start(t[:], hTv[:, 2 * c:2 * c + 2, 0:1024])
                        hts[(0, c)] = t
                    nc.sync.dma_start(wq_sb[:, 1024:4096], wqT[:, 1024:4096])
                    nc.sync.dma_start(wq_sb[:, 4096:2 * 4096], wqT[:, 4096:2 * 4096])
                    nc.sync.dma_start(wq_sb[:, 2 * 4096:3 * 4096],
                                      wqT[:, 2 * 4096:3 * 4096])
                    nc.sync.dma_start(wq_sb[:, 3 * 4096:4 * 4096],
                                      wqT[:, 3 * 4096:4 * 4096])
                    # needed from first rope evict (~30us in)
                    nc.sync.dma_start(cos_sb[0:64, :], cosT)
                    nc.sync.dma_start(cos_sb[64:128, :], cosT)
                    nc.sync.dma_start(sin_sb[0:64, :], sinT)
                    nc.sync.dma_start(sin_sb[64:128, :], sinT)
                    nc.sync.dma_start(ones_sb[:], ones_c)


                    def wslice(m, kt):
                        if m < HQL:
                            return wq_sb[:, kt * 512 + m * 128: kt * 512 + (m + 1) * 128]
                        if m == HQL:
                            return wk_sb[:, kt * 128:(kt + 1) * 128]
                        return wv_sb[:, kt * 128:(kt + 1) * 128]

                    def rope_evict(dst_ap, src_sb, qoff):
                        """rope from f32 SBUF tile [128,512] -> dst bf16 [128,512]."""
                        cs = cos_sb[:, qoff:qoff + 512]
                        sn = sin_sb[:, qoff:qoff + 512]
                        t1 = epool.tile([64, 512], F32, tag="t1")
                        t2 = epool.tile([64, 512], F32, tag="t2")
                        t3 = epool.tile([64, 512], F32, tag="t3")
                        t4 = epool.tile([64, 512], F32, tag="t4")
                        nc.vector.tensor_tensor(t1[:], src_sb[0:64, :], cs[0:64, :], ALU.mult)
                        nc.vector.tensor_tensor(t2[:], src_sb[64:128, :], sn[64:128, :], ALU.mult)
                        nc.vector.tensor_tensor(dst_ap[0:64, :], t1[:], t2[:], ALU.subtract)
                        nc.gpsimd.tensor_tensor(t3[:], src_sb[64:128, :], cs[64:128, :], ALU.mult)
                        nc.gpsimd.tensor_tensor(t4[:], src_sb[0:64, :], sn[0:64, :], ALU.mult)
                        nc.gpsimd.tensor_tensor(dst_ap[64:128, :], t3[:], t4[:], ALU.add)

                    # h tiles per (half, kt): [128, 1024]
                    def evict(m, quarter, acc):
                        b, qoff = quarter, quarter * 512
                        if m < 5:
                            asb = epool.tile([P, 512], F32, tag="asb")
                            nc.scalar.activation(asb[:], acc[:], AF.Copy)
                            if m < HQL:
                                dst = q_sb[:, b * (HQL * QO) + m * QO:
                                           b * (HQL * QO) + (m + 1) * QO]
                            else:
                                dst = kn_sb[:, qoff:qoff + 512]
                            rope_evict(dst, asb, qoff)
                        else:
                            vt = epool.tile([P, 512], BF16, tag="vt")
                            nc.scalar.activation(vt[:], acc[:], AF.Copy)
                            tp = tpool.tile([P, 4, P], BF16, tag="tp", name=f"tp{quarter}")
                            for t in range(4):
                                nc.tensor.transpose(
                                    tp[:, t, :], vt[:, t * 128:(t + 1) * 128],
                                    ident[:])
                            nc.vector.tensor_copy(
                                vn_sb[:, qoff:qoff + 512], tp[:, :, :])

                    # v (m=5) and k (m=4) first so the PE transposes and
                    # vector copies clear PSUM well before the phase ends
                    MORD = [5, 4, 0, 1, 2, 3]
                    for half in range(2):
                        if half == 0:
                            pass
                        else:
                            # h half1: emitted after half0's consumers so the
                            # pool-slot WAR chain is valid; queued behind the
                            # first wave on both queues
                            for c in range(16):
                                t = hpool.tile([P, 2 * 1024], BF16, tag="h",
                                               name=f"h1_{c}")
                                (nc.gpsimd if c % 2 == 0 else nc.scalar).dma_start(
                                    t[:], hTv[:, 2 * c:2 * c + 2, 1024:2048])
                                hts[(1, c)] = t
                            # ctx tensors: needed only from attention onward
                            nc.gpsimd.dma_start(kctx_sb[:], kctxT)
                            nc.gpsimd.dma_start(vctx_sb[:], vctx)
                            nc.sync.dma_start(msk_sb[:], msk)
                        for qq in range(2):
                            quarter = half * 2 + qq
                            if quarter == 0:
                                # m-inner: consume each (w,h) kt-tile for all
                                # 6 outputs as soon as its DMA lands
                                accs = [apool.tile([P, 512], F32, tag="acc",
                                                   name=f"acc0_{m}") for m in range(6)]
                                for kt in range(KH):
                                    for m in MORD:
                                        nc.tensor.matmul(
                                            accs[m][:], wslice(m, kt),
                                            hts[(half, kt // 2)][:, (kt % 2) * 1024 + qq * 512:
                                                (kt % 2) * 1024 + (qq + 1) * 512],
                                            start=(kt == 0), stop=(kt == KH - 1))
                                for m in MORD:
                                    evict(m, quarter, accs[m])
                            else:
                                for m in MORD:
                                    acc = apool.tile([P, 512], F32, tag="acc",
                                                     name=f"acc{quarter}_{m}")
                                    for kt in range(KH):
                                        nc.tensor.matmul(
                                            acc[:], wslice(m, kt),
                                            hts[(half, kt // 2)][:, (kt % 2) * 1024 + qq * 512:
                                                (kt % 2) * 1024 + (qq + 1) * 512],
                                            start=(kt == 0), stop=(kt == KH - 1))
                                    evict(m, quarter, acc)

                # ============ Phase B: attention ============
                with tc.tile_pool(name="wosb", bufs=1) as wopool, \
                     tc.tile_pool(name="ovstream", bufs=8) as ovpool:
                    woc_sb = wopool.tile([P, KH * QO], BF16)  # (kt,ob) at kt*512+ob*128
                    nc.sync.dma_start(
                        woc_sb[:], woTc)
                    ovs = {}

                    def prefetch_ovs(bp):
                        # 8-kt chunks per batch: [128, 8*512] each, 4 per batch
                        for bb in (2 * bp, 2 * bp + 1):
                            for c in range(4):
                                ov = ovpool.tile([P, 8 * QO], BF16, tag="ov",
                                                 name=f"ov{bb}_{c}")
                                dq = (nc.sync, nc.gpsimd)[c % 2]
                                dq.dma_start(
                                    ov[:], oouts[bb][c * 1024:(c + 1) * 1024, :]
                                    .rearrange("(k p) q -> p k q", p=P))
                                ovs[(bb, c)] = ov

                    def ovslice(bb, kt):
                        return ovs[(bb, kt // 8)][:, (kt % 8) * QO:(kt % 8 + 1) * QO]

                    with tc.tile_pool(name="spsum", bufs=2, space="PSUM") as spool, \
                         tc.tile_pool(name="opsum", bufs=1, space="PSUM") as opool, \
                         tc.tile_pool(name="dpsum", bufs=1, space="PSUM") as dpool, \
                         tc.tile_pool(name="ptile", bufs=8) as ppool, \
                         tc.tile_pool(name="rtile", bufs=2) as rpool, \
                         tc.tile_pool(name="osb", bufs=4) as osbpool:
                        LAG = 6  # PV/ones trail scores/exp by 6 chunks

                        for b in range(B):
                            for hg in range(2):
                                po = opool.tile([P, 1024], F32, tag="po", name=f"po{b}_{hg}")
                                pd = dpool.tile([P, 1024], F32, tag="pd", name=f"pd{b}_{hg}")
                                qbase = b * (HQL * QO) + hg * 1024
                                pts = [None] * KC

                                def pv_ones(c):
                                    # new-kv chunk jn only attends q >= jn*128
                                    off = 0 if c < CC else (c - CC) * 128
                                    if c < CC:
                                        vsl = vctx_sb[:, (b * CC + c) * 128:
                                                      (b * CC + c + 1) * 128]
                                    else:
                                        jn = c - CC
                                        vsl = vn_sb[:, (b * 4 + jn) * 128:
                                                    (b * 4 + jn + 1) * 128]
                                    st_, sp_ = (c == 0), (c == KC - 1)
                                    for i in range(2):
                                        nc.tensor.matmul(
                                            po[:, i * 512 + off:(i + 1) * 512], vsl,
                                            pts[c][:, i * 512 + off:(i + 1) * 512],
                                            start=st_, stop=sp_)
                                    for i in range(2):
                                        nc.tensor.matmul(
                                            pd[:, i * 512 + off:(i + 1) * 512], ones_sb[:],
                                            pts[c][:, i * 512 + off:(i + 1) * 512],
                                            start=st_, stop=sp_)

                                for c in range(KC):
                                    st = spool.tile([P, 1024], F32, tag="st", name=f"st{c%2}")
                                    pt = ppool.tile([P, 1024], BF16, tag="pt", name=f"pt{c%8}")
                                    if c < CC:
                                        kl = kctx_sb[:, b * ctxl + c * 128:
                                                     b * ctxl + (c + 1) * 128]
                                        for i in range(2):
                                            nc.tensor.matmul(
                                                st[:, i * 512:(i + 1) * 512], kl,
                                                q_sb[:, qbase + i * 512: qbase + (i + 1) * 512],
                                                start=True, stop=True)
                                        nc.scalar.activation(pt[:], st[:], AF.Exp,
                                                             scale=SM_SCALE)
                                    else:
                                        jn = c - CC
                                        off = jn * 128
                                        kl = kn_sb[:, b * QO + jn * 128:
                                                   b * QO + (jn + 1) * 128]
                                        for i in range(2):
                                            nc.tensor.matmul(
                                                st[:, i * 512 + off:(i + 1) * 512], kl,
                                                q_sb[:, qbase + i * 512 + off:
                                                      qbase + (i + 1) * 512],
                                                start=True, stop=True)
                                        # mask only the diagonal 128-col block
                                        for i in range(2):
                                            nc.vector.tensor_tensor(
                                                st[:, i * 512 + off: i * 512 + off + 128],
                                                st[:, i * 512 + off: i * 512 + off + 128],
                                                msk_sb[:, jn * QO + off:
                                                       jn * QO + off + 128],
                                                ALU.add)
                                        if jn == 0:
                                            nc.scalar.activation(pt[:], st[:], AF.Exp,
                                                                 scale=SM_SCALE)
                                        else:
                                            for i in range(2):
                                                nc.scalar.activation(
                                                    pt[:, i * 512 + off:(i + 1) * 512],
                                                    st[:, i * 512 + off:(i + 1) * 512],
                                                    AF.Exp, scale=SM_SCALE)
                                    pts[c] = pt
                                    if c >= LAG:
                                        pv_ones(c - LAG)
                                for c in range(KC - LAG, KC):
                                    pv_ones(c)
                                rsb = rpool.tile([P, 1024], F32, tag="rsb")
                                nc.vector.reciprocal_approx_fast(rsb[:], pd[:])
                                ot = osbpool.tile([P, 1024], BF16)
                                nc.vector.tensor_tensor(ot[:], po[:], rsb[:], ALU.mult)
                                for i in range(2):
                                    m = hg * 2 + i
                                    nc.sync.dma_start(
                                        oins[b][m * 128:(m + 1) * 128, :],
                                        ot[:, i * 512:(i + 1) * 512])
                            nc.gpsimd.collective_compute(
                                "AllGather", ALU.bypass, replica_groups=rg,
                                ins=[oins[b][:]], outs=[oouts[b][:]])
                            # prefetch AG results for o_proj as soon as ready
                            if b == 1:
                                prefetch_ovs(0)

                    # ============ Phase C: column-parallel o_proj ============
                    with tc.tile_pool(name="cpsum", bufs=4, space="PSUM") as cpool, \
                         tc.tile_pool(name="outsb", bufs=3) as outpool:
                        def oproj_evict(bp, ob, pc):
                            ot2 = outpool.tile([P, 1024], BF16)
                            nc.vector.tensor_copy(ot2[:], pc[:])
                            for i in range(2):
                                bb = 2 * bp + i
                                nc.sync.dma_start(
                                    outT[ob * 128:(ob + 1) * 128,
                                         bb * QO:(bb + 1) * QO],
                                    ot2[:, i * 512:(i + 1) * 512])

                        # bp0: kt-outer (consumes ovs chunks as they land,
                        # frees them early for bp1's prefetch)
                        pcs = [cpool.tile([P, 1024], F32, tag="pc", name=f"pc0_{ob}")
                               for ob in range(QO // 128)]
                        for kt in range(KH):
                            for ob in range(QO // 128):
                                for i in range(2):
                                    nc.tensor.matmul(
                                        pcs[ob][:, i * 512:(i + 1) * 512],
                                        woc_sb[:, kt * 512 + ob * 128:
                                               kt * 512 + (ob + 1) * 128],
                                        ovslice(i, kt),
                                        start=(kt == 0), stop=(kt == KH - 1))
                        prefetch_ovs(1)
                        for ob in range(QO // 128):
                            oproj_evict(0, ob, pcs[ob])
                        # bp1: ob-outer so each accumulator's evict+DMA
                        # overlaps the next ob's matmuls
                        for ob in range(QO // 128):
                            pc = cpool.tile([P, 1024], F32, tag="pc", name=f"pc1_{ob}")
                            for kt in range(KH):
                                for i in range(2):
                                    nc.tensor.matmul(
                                        pc[:, i * 512:(i + 1) * 512],
                                        woc_sb[:, kt * 512 + ob * 128:
                                               kt * 512 + (ob + 1) * 128],
                                        ovslice(2 + i, kt),
                                        start=(kt == 0), stop=(kt == KH - 1))
                            oproj_evict(1, ob, pc)
    nc.compile()
    return nc


_NC_CACHE = {}


def _get_program(ctxl):
    if ctxl not in _NC_CACHE:
        _NC_CACHE[ctxl] = build_program(ctxl)
    return _NC_CACHE[ctxl]


def run(inputs, trace=False):
    in_maps, ctxl = host_prep(inputs)
    nc = _get_program(ctxl)
    kw = dict(tmpdir="/tmp/trace_out") if trace else {}
    res = run_bass_kernel_spmd(nc, in_maps, core_ids=list(range(NCORES)), trace=trace, **kw)
    out = np.empty((N, HID), np.float32)
    for i, r in enumerate(res.results):
        out[:, i * QO:(i + 1) * QO] = np.asarray(r["outT"]).T.astype(np.float32)
    return out, res


def kernel(**inputs) -> np.ndarray:
    out, _ = run(inputs, trace=False)
    return out
